# revision 33
# baseline (speedup 1.0000x reference)
"""Trainium2 Bass kernel for pairwise-GEMM + 8-bin histc + L2 normalize.

Strategy (hardcoded for bs=64, F=256, M=128, BINS=8, 8 cores):
  - scores[a,b] = matf[a]^T @ matf[b]; histogram over all M*M elements is
    invariant under transpose, so only unique diagonals are computed:
    for each a, d=0..32, b=(a+d)%64.  Each core owns 16 (matf, a) slots
    (2 matfs x 2 windows x 4 a-values) -> 528 pairs/core.
  - fp16 GEMM (1 cyc/row on PE) in 512-col chunks -> PSUM fp32.
  - ScalarE (plus VectorE during the pipeline-fill phase) evacuates each
    chunk PSUM->SBUF as fp16.
  - DMA bounces the fp16 scores through DRAM scratch regions laid out
    PAIR-MAJOR: row u (= slot*33+d) holds that pair's 16384 elements.
  - Big tiles [128 pairs, 16384] load back to SBUF.  Per-pair row stats
    (min/max via a TT tree at 2x + small reduce) feed per-partition
    thresholds.  Counting exploits that plain tensor_scalar runs 4x on
    packed fp16 while ANY accumulate runs 1x: thresholds 1-4 compute a
    0/1 mask at 4x, then the TensorE sums it (identity-matmul PSUM
    accumulation over 32 aliased 512-col chunks) with a cheap ScalarE
    final reduce; threshold 5 sums its mask with a TT-add tree on
    VectorE; thresholds 6-7 run directly on ScalarE via Sign+accum.
  - Tail (last 16 pairs): loaded as 128 sub-rows of 2048; per-pair stats
    rebuilt across 8-partition groups via PE transpose + selector
    matmul; counts via direct 1x accumulates (small).
  - Host: counts -> bins -> L2 normalize -> mirror to (b,a).
"""

import os
import sys

for _p in ("/opt/trn_rl_repo", "/root/.axon_site/_ro/trn_rl_repo"):
    if os.path.isdir(_p) and _p not in sys.path:
        sys.path.insert(0, _p)

import numpy as np

import concourse.bacc as bacc
import concourse.mybir as mybir
from concourse import bass_utils
from concourse.tile import TileContext

F32 = mybir.dt.float32
F16 = mybir.dt.float16
AF = mybir.ActivationFunctionType
ALU = mybir.AluOpType
AX = mybir.AxisListType

BS = 64          # batch (a/b index range)
FD = 256         # feature dim (contraction)
M = 128          # matrix dim -> partition dim
BINS = 8
NDIAG = 33       # d = 0..32 unique diagonals per a
WINA = 4         # a-values per window
NWIN = 2         # windows per core
NMATF = 2
NSLOT = NMATF * NWIN * WINA             # 16 (matf, a) slots per core
NPHASE = NMATF * NWIN                   # 4 window phases of 4 slots
WCOLS = (WINA - 1 + NDIAG) * M          # 36*128 = 4608 columns per window slice
NPAIR = NSLOT * NDIAG                   # 528 pairs per core
NBIG = NPAIR // 128                     # 4 full [128, 16384] tiles
NTAIL = NPAIR - NBIG * 128              # 16 pairs in the tail tile
TAILSUB = 8                             # tail pairs split into 8 sub-rows
TAILW = (M * M) // TAILSUB              # 2048
N_CHUNK = (NDIAG * M + 511) // 512      # 9 rhs chunks per a (8x512 + 1x128)
NELEM = float(M * M)                    # 16384 elements per pair

PE_JS = (1, 2, 3, 4)   # mask at 4x + TensorE chunk-accum + ACT final reduce
TREE_JS = (5,)         # mask at 4x + VectorE TT-add tree
ACT_JS = (6, 7)        # ScalarE Sign+accum, four quarters each
OUTC = 16  # out row: c1..c5 -> 0..4; S6 quarters 6..9; S7 quarters 10..13;
           # pmin 14, pmax 15

_CACHED = {}


def _core_bases(c):
    """(window0 base a, window1 base a) for core c."""
    return 4 * c, 60 - 4 * c


def _build_nc():
    nc = bacc.Bacc(
        "TRN2", target_bir_lowering=False, debug=False, enable_asserts=False
    )
    xw_d = nc.dram_tensor(
        "xw", [NPHASE, 2, M, WCOLS], F16, kind="ExternalInput"
    )
    ident_d = nc.dram_tensor("ident", [M, M], F32, kind="ExternalInput")
    ident16_d = nc.dram_tensor("ident16", [M, M], F16, kind="ExternalInput")
    selq_d = nc.dram_tensor("selq", [NTAIL, M], F32, kind="ExternalInput")
    out_d = nc.dram_tensor(
        "out", [NBIG * 128 + NTAIL * TAILSUB, OUTC], F32, kind="ExternalOutput"
    )

    with TileContext(nc) as tc:
        with (
            tc.tile_pool(name="win", bufs=2) as win_pool,
            tc.tile_pool(name="const", bufs=1) as const_pool,
            tc.tile_pool(name="stage", bufs=6) as stage_pool,
            tc.tile_pool(name="btile", bufs=2) as btile_pool,
            tc.tile_pool(name="junk", bufs=4) as junk_pool,
            tc.tile_pool(name="junka", bufs=1) as junka_pool,
            tc.tile_pool(name="small", bufs=2) as small_pool,
            tc.tile_pool(name="cnt", bufs=2) as cnt_pool,
            tc.tile_pool(name="pg", bufs=4, space="PSUM") as psum_g,
            tc.tile_pool(name="pc", bufs=3, space="PSUM") as psum_c,
            tc.tile_pool(name="ps", bufs=1, space="PSUM") as psum_s,
            tc.tile_pool(name="dscr", bufs=1, space="DRAM") as dram_pool,
        ):
            ident = const_pool.tile([M, M], F32, tag="ident")
            nc.sync.dma_start(ident[:], ident_d[:])
            ident16 = const_pool.tile([M, M], F16, tag="ident16")
            nc.sync.dma_start(ident16[:], ident16_d[:])
            selq = const_pool.tile([NTAIL, M], F32, tag="selq")
            nc.sync.dma_start(selq[:], selq_d[:])

            # DRAM scratch, one region per output tile (rows of `scores`)
            regions = []
            for t in range(NBIG + 1):
                nrow = NTAIL if t == NBIG else 128
                regions.append(
                    dram_pool.tile(
                        [nrow, M * M], F16, tag=f"reg{t}", name=f"reg{t}"
                    )
                )

            def reg_rows(u0, n):
                """Split row range [u0, u0+n) by region; yields
                (region_tile, local_row0, nrows, global_row0)."""
                u = u0
                while u < u0 + n:
                    r = min(u // 128, NBIG)
                    base = r * 128
                    hi = min(u0 + n, base + (NTAIL if r == NBIG else 128))
                    yield regions[r], u - base, hi - u, u
                    u = hi

            junk_a = junka_pool.tile([M, (M * M) // 4], F16, tag="ja")
            junk512 = junka_pool.tile([M, 512], F16, tag="j512")
            tscr = junka_pool.tile([M, (M * M) // 2], F16, tag="tscr")

            # ---- helpers -----------------------------------------------
            def tt_tree(src_ap, total, op, out_col):
                """Row-reduce src_ap [128, total] by `op` via a TT tree
                (2x fp16): level 0 into tscr, then strict in-place halving
                (out aliases in0; the DVE stream reads each position before
                writing it), then one small reduce into out_col [P, 1]."""
                h = total // 2
                nc.vector.tensor_tensor(
                    tscr[:, 0:h], src_ap[:, 0:h], src_ap[:, h : 2 * h], op
                )
                while h > 512:
                    nh = h // 2
                    nc.vector.tensor_tensor(
                        tscr[:, 0:nh], tscr[:, 0:nh], tscr[:, nh:h], op
                    )
                    h = nh
                nc.vector.tensor_reduce(out_col, tscr[:, 0:h], AX.X, op)

            def thresholds(pmin, pmax, thr, nthr, np_=M):
                delta = small_pool.tile([M, 1], F32, tag="delta")
                nc.vector.tensor_tensor(
                    delta[:np_], pmax[:np_], pmin[:np_], ALU.subtract
                )
                for j in range(1, BINS):
                    nc.vector.tensor_scalar(
                        thr[:np_, j - 1 : j], delta[:np_], float(j) / BINS,
                        pmin[:np_], ALU.mult, ALU.add,
                    )
                nc.vector.tensor_scalar(
                    nthr[:np_], thr[:np_], -1.0, None, ALU.mult
                )

            # ---- big-tile compute --------------------------------------
            def big_tile(t):
                r0 = t * 128
                bt = btile_pool.tile([M, M * M], F16, tag="bt")
                nc.scalar.dma_start(bt[:], regions[t][:])
                pstat = small_pool.tile([M, 2], F32, tag="pstat")
                pmin, pmax = pstat[:, 0:1], pstat[:, 1:2]
                tt_tree(bt, M * M, ALU.max, pmax)
                tt_tree(bt, M * M, ALU.min, pmin)
                thr = small_pool.tile([M, BINS - 1], F32, tag="thr")
                nthr = small_pool.tile([M, BINS - 1], F32, tag="nthr")
                thresholds(pmin, pmax, thr, nthr)
                cnt = cnt_pool.tile([M, OUTC], F32, tag="cnt")
                # ScalarE-direct thresholds (Sign + accum, four quarters)
                for j in ACT_JS:
                    c0 = 6 if j == 6 else 10
                    q = (M * M) // 4
                    for hh in range(4):
                        nc.scalar.activation(
                            junk_a[:], bt[:, hh * q : (hh + 1) * q],
                            AF.Sign, bias=nthr[:, j - 1 : j],
                            accum_out=cnt[:, c0 + hh : c0 + hh + 1],
                        )
                # mask at 4x (two half-width instrs per threshold, 4-slot
                # ring); sum on TensorE (chunk-accum) or VectorE TT tree
                half = (M * M) // 2
                for j in PE_JS + TREE_JS:
                    s0 = junk_pool.tile(
                        [M, half], F16, tag="jd", name=f"m{j}a"
                    )
                    s1 = junk_pool.tile(
                        [M, half], F16, tag="jd", name=f"m{j}b"
                    )
                    nc.vector.tensor_scalar(
                        s0[:], bt[:, :half], thr[:, j - 1 : j], None,
                        ALU.is_lt,
                    )
                    nc.vector.tensor_scalar(
                        s1[:], bt[:, half:], thr[:, j - 1 : j], None,
                        ALU.is_lt,
                    )
                    if j in TREE_JS:
                        h = half
                        nc.vector.tensor_tensor(
                            tscr[:, 0:h], s0[:], s1[:], ALU.add
                        )
                        while h > 512:
                            nh = h // 2
                            nc.vector.tensor_tensor(
                                tscr[:, 0:nh], tscr[:, 0:nh],
                                tscr[:, nh:h], ALU.add,
                            )
                            h = nh
                        nc.vector.tensor_reduce(
                            cnt[:, j - 1 : j], tscr[:, 0:h], AX.X, ALU.add
                        )
                    else:
                        cb = psum_c.tile([M, 512], F32, tag="cb")
                        for ch in range(32):
                            src = s0 if ch < 16 else s1
                            c2 = ch % 16
                            nc.tensor.matmul(
                                cb[:], ident16[:],
                                src[:, c2 * 512 : (c2 + 1) * 512],
                                start=(ch == 0), stop=(ch == 31),
                            )
                        nc.scalar.activation(
                            junk512[:], cb[:], AF.Copy,
                            accum_out=cnt[:, j - 1 : j],
                        )
                nc.vector.tensor_copy(cnt[:, 14:15], pmin)
                nc.vector.tensor_copy(cnt[:, 15:16], pmax)
                nc.gpsimd.dma_start(out_d[r0 : r0 + 128, :], cnt[:])

            # ---- tail-tile compute (16 pairs as 128 sub-rows of 2048) --
            def tail_tile():
                tt = btile_pool.tile([M, TAILW], F16, tag="tt")
                nc.scalar.dma_start(
                    tt[:],
                    regions[NBIG][:].rearrange(
                        "u (s n) -> (u s) n", s=TAILSUB
                    ),
                )
                smm = small_pool.tile([M, 2], F32, tag="smm")
                nc.vector.tensor_reduce(smm[:, 0:1], tt[:], AX.X, ALU.min)
                nc.vector.tensor_reduce(smm[:, 1:2], tt[:], AX.X, ALU.max)
                # combine sub-row stats across 8-partition groups with a
                # single MAX by negating the min column first:
                # [128,2] -T-> [2,128] -> seg-max -> [2,16] -T-> [16,2]
                # -> thresholds [16,7] -> selq matmul broadcast -> [128,7]
                smm2 = small_pool.tile([M, 2], F32, tag="smm2")
                nc.vector.tensor_scalar(
                    smm2[:, 0:1], smm[:, 0:1], -1.0, None, ALU.mult
                )
                nc.vector.tensor_copy(smm2[:, 1:2], smm[:, 1:2])
                tp = psum_s.tile([M, M], F32, tag="ps1")
                nc.tensor.transpose(tp[:2, :M], smm2[:], ident[:])
                row2 = small_pool.tile([2, M], F32, tag="row2")
                nc.vector.tensor_copy(row2[:], tp[:2, :M])
                st16 = small_pool.tile([2, NTAIL], F32, tag="st16")
                rv = row2.rearrange("p (q s) -> p q s", q=NTAIL)
                nc.vector.tensor_reduce(st16[:], rv[:], AX.X, ALU.max)
                tps = psum_s.tile([M, M], F32, tag="ps1", name="tps")
                nc.tensor.transpose(tps[:NTAIL, :2], st16[:], ident[:2, :2])
                # pm16 col0 = -pmin, col1 = pmax (per tail pair)
                pm16 = small_pool.tile([NTAIL, 2], F32, tag="pm16")
                nc.vector.tensor_copy(pm16[:], tps[:NTAIL, :2])
                thr16 = small_pool.tile([NTAIL, BINS - 1], F32, tag="thr16")
                d16 = small_pool.tile([NTAIL, 1], F32, tag="d16")
                p16 = small_pool.tile([NTAIL, 1], F32, tag="p16")
                nc.vector.tensor_tensor(
                    d16[:], pm16[:, 1:2], pm16[:, 0:1], ALU.add
                )
                nc.vector.tensor_scalar(
                    p16[:], pm16[:, 0:1], -1.0, None, ALU.mult
                )
                for j in range(1, BINS):
                    nc.vector.tensor_scalar(
                        thr16[:, j - 1 : j], d16[:], float(j) / BINS,
                        p16[:], ALU.mult, ALU.add,
                    )
                thrb = psum_s.tile([M, M], F32, tag="ps1", name="thrb")
                nc.tensor.matmul(thrb[:, : BINS - 1], selq[:], thr16[:])
                thrT = small_pool.tile([M, BINS - 1], F32, tag="thrT")
                nthrT = small_pool.tile([M, BINS - 1], F32, tag="nthrT")
                nc.vector.tensor_copy(thrT[:], thrb[:, : BINS - 1])
                nc.vector.tensor_scalar(
                    nthrT[:], thrT[:], -1.0, None, ALU.mult
                )
                cnt = cnt_pool.tile([M, OUTC], F32, tag="cnt")
                jd = junk_pool.tile([M, TAILW], F16, tag="jd")
                for j in PE_JS + TREE_JS:
                    nc.vector.tensor_scalar(
                        jd[:], tt[:], thrT[:, j - 1 : j], None,
                        ALU.is_lt, ALU.add,
                        accum_out=cnt[:, j - 1 : j],
                    )
                q = TAILW // 4
                for j in ACT_JS:
                    c0 = 6 if j == 6 else 10
                    for hh in range(4):
                        nc.scalar.activation(
                            junk_a[:, :q], tt[:, hh * q : (hh + 1) * q],
                            AF.Sign, bias=nthrT[:, j - 1 : j],
                            accum_out=cnt[:, c0 + hh : c0 + hh + 1],
                        )
                nc.vector.tensor_copy(cnt[:, 14:15], smm[:, 0:1])
                nc.vector.tensor_copy(cnt[:, 15:16], smm[:, 1:2])
                nc.gpsimd.dma_start(out_d[NBIG * 128 :, :], cnt[:])

            # ---- main loop: GEMM + evac + scatter-store ----------------
            # Emission order: slot 15 first (fills the tail DRAM region so
            # tail_tile's long serial glue overlaps the big-tile pipeline),
            # then slots 0..14.  Window phases: 3, 0, 1, 2, 3 (reload).
            def load_win(ph):
                w = win_pool.tile([M, 2 * WCOLS], F16, tag="win")
                nc.sync.dma_start(w[:, :WCOLS], xw_d[ph, 0])
                nc.sync.dma_start(w[:, WCOLS:], xw_d[ph, 1])
                return w

            def do_slot(slot, win, emit_idx):
                ph, al = divmod(slot, WINA)
                wk0 = win[:, :WCOLS]
                wk1 = win[:, WCOLS:]
                lhs0 = wk0[:, al * M : (al + 1) * M]
                lhs1 = wk1[:, al * M : (al + 1) * M]
                c0 = al * M
                for ch in range(N_CHUNK):
                    n = min(512, NDIAG * M - ch * 512)
                    lo = c0 + ch * 512
                    bank = psum_g.tile([M, 512], F32, tag="bank")
                    nc.tensor.matmul(
                        bank[:, :n], lhs0, wk0[:, lo : lo + n],
                        start=True, stop=False,
                    )
                    nc.tensor.matmul(
                        bank[:, :n], lhs1, wk1[:, lo : lo + n],
                        start=False, stop=True,
                    )
                    st = stage_pool.tile([M, 512], F16, tag="st")
                    if emit_idx < 5 and ch % 2 == 1:
                        nc.vector.tensor_copy(st[:, :n], bank[:, :n])
                    else:
                        nc.scalar.copy(st[:, :n], bank[:, :n])
                    u0 = slot * NDIAG + ch * 4
                    npair = n // M
                    for reg, lr0, nr, g0 in reg_rows(u0, npair):
                        soff = (g0 - u0) * M
                        nc.sync.dma_start(
                            reg[lr0 : lr0 + nr, :].rearrange(
                                "u (p n) -> p u n", p=M
                            ),
                            st[:, soff : soff + nr * M].rearrange(
                                "p (u n) -> p u n", u=nr
                            ),
                        )

            # Emit each slot-group one step AHEAD of the tile that consumes
            # the previous group, so evac/stores/loads overlap tile compute.
            phase_seq = [3, 0, 1, 2, 3]
            slot_seq = [[15], [0, 1, 2, 3], [4, 5, 6, 7], [8, 9, 10, 11],
                        [12, 13, 14]]
            after_group = [
                [],
                [tail_tile],
                [lambda: big_tile(0)],
                [lambda: big_tile(1)],
                [lambda: big_tile(2), lambda: big_tile(3)],
            ]
            win = load_win(phase_seq[0])
            emit_idx = 0
            for pi in range(len(phase_seq)):
                if pi + 1 < len(phase_seq):
                    win_next = load_win(phase_seq[pi + 1])
                else:
                    win_next = None
                for slot in slot_seq[pi]:
                    do_slot(slot, win, emit_idx)
                    emit_idx += 1
                for fn in after_group[pi]:
                    fn()
                win = win_next
    nc.compile()
    return nc


def _host_inputs():
    ident = np.eye(M, dtype=np.float32)
    ident16 = np.eye(M, dtype=np.float16)
    selq = np.zeros((NTAIL, M), dtype=np.float32)
    for q in range(NTAIL):
        selq[q, q * TAILSUB : (q + 1) * TAILSUB] = 1.0
    return ident, ident16, selq


def _install_ntff_hook():
    """boot() skips NTFF hook registration when antenv.axon_hooks is
    missing at interpreter start; install it lazily here."""
    try:
        from antenv.axon_hooks import (
            get_axon_ntff_profile_hook,
            set_axon_ntff_profile_hook,
        )
        if get_axon_ntff_profile_hook() is None:
            from trn_agent_boot.trn_boot import _ntff_profile_via_ctypes
            set_axon_ntff_profile_hook(
                _ntff_profile_via_ctypes("/opt/axon/libaxon_pjrt.so")
            )
    except Exception as e:
        print("ntff hook install failed:", e)


def profile_exec_ns(tmpdir="/tmp/bass_hist_trace"):
    """Re-run the last kernel() invocation with NTFF tracing; returns ns."""
    if "nc" not in _CACHED or "in_maps" not in _CACHED:
        return None
    _install_ntff_hook()
    import shutil

    shutil.rmtree(tmpdir, ignore_errors=True)
    os.makedirs(tmpdir, exist_ok=True)
    res = bass_utils.run_bass_kernel_spmd(
        _CACHED["nc"], _CACHED["in_maps"], list(range(8)),
        trace=True, tmpdir=tmpdir,
    )
    return res.exec_time_ns


def kernel(matf1: np.ndarray, matf2: np.ndarray) -> np.ndarray:
    if "nc" not in _CACHED:
        _CACHED["nc"] = _build_nc()
    nc = _CACHED["nc"]
    ident, ident16, selq = _host_inputs()

    xs = []
    for matf in (matf1, matf2):
        X = np.ascontiguousarray(
            np.asarray(matf, dtype=np.float32).transpose(1, 0, 2)
        ).reshape(FD, BS * M)
        xs.append(
            np.concatenate([X, X[:, : (NDIAG + WINA) * M]], axis=1).astype(
                np.float16
            )
        )

    in_maps = []
    for c in range(8):
        b0, b1 = _core_bases(c)
        xw = np.empty((NPHASE, 2, M, WCOLS), dtype=np.float16)
        for m in range(NMATF):
            for w, base in enumerate((b0, b1)):
                ph = m * NWIN + w
                sl = xs[m][:, base * M : base * M + WCOLS]
                xw[ph, 0] = sl[:M]
                xw[ph, 1] = sl[M:]
        in_maps.append(
            {"xw": xw, "ident": ident, "ident16": ident16, "selq": selq}
        )

    _CACHED["in_maps"] = in_maps
    res = bass_utils.run_bass_kernel_spmd(nc, in_maps, list(range(8)))

    N = NELEM
    hist = np.zeros((NMATF, BS, BS, BINS), dtype=np.float64)
    for c in range(8):
        out = np.asarray(res.results[c]["out"], dtype=np.float64)
        b0, b1 = _core_bases(c)
        for u in range(NPAIR):
            slot, d = divmod(u, NDIAG)
            ph, al = divmod(slot, WINA)
            m, w = divmod(ph, NWIN)
            a = (b0 if w == 0 else b1) + al
            b = (a + d) % BS
            if u < NBIG * 128:
                row = out[u]
                pmn, pmx = row[14], row[15]
            else:
                q = u - NBIG * 128
                rows = out[
                    NBIG * 128 + q * TAILSUB : NBIG * 128 + (q + 1) * TAILSUB
                ]
                row = rows.sum(axis=0)
                pmn, pmx = rows[:, 14].min(), rows[:, 15].max()
            cum = np.zeros(BINS - 1)
            for j in range(1, BINS):
                if j in ACT_JS:
                    c0 = 6 if j == 6 else 10
                    s = row[c0 : c0 + 4].sum()
                    cum[j - 1] = np.floor((N - s) / 2)
                else:
                    cum[j - 1] = row[j - 1]
            bins = np.empty(BINS)
            if pmx <= pmn:
                bins[:] = 0.0
                bins[0] = N
            else:
                bins[0] = cum[0]
                bins[1:7] = np.diff(cum)
                bins[7] = N - cum[6]
            hist[m, a, b] = bins
            hist[m, b, a] = bins

    counts = hist.astype(np.float32).reshape(NMATF, BS * BS, BINS)
    norm = np.linalg.norm(counts, axis=-1, keepdims=True)
    h = counts / np.maximum(norm, 1e-12)
    return np.concatenate([h[0], h[1]], axis=-1)
